# revision 24
# baseline (speedup 1.0000x reference)
"""Cantor global attention kernel for Trainium2 (8 NeuronCores, SPMD).

Strategy: data-parallel over batch B=64 -> 8 cores x 8 rows each.
Per core, every expert slab [8, 4096] is flattened to SBUF [128, 256]
(partition = b*16 + p//256, col = p%256); experts sit side by side in
the free dimension, grouped 4 per tile.  The W=3 neighbor gather and
the beta/temperature gating are folded into per-(e,w) instruction
operand offsets and exp-activation scale immediates, baked at build
time from the runtime routes/betas/temperature values (tiny [16,3]
control-plane tensors).

Engine placement (per core, f32).  DVE 2-operand ops and GpSimd ops
serialize on the shared SBUF port pair, so GpSimd runs NO tensor ops -
only SWDGE descriptor generation:
  - projection averaging (all of Q,K,V): DMA-accumulate (CCE add in
    the SDMA engines) - zero compute-engine cost
  - t_w = Qs*Ks:      DVE tensor_mul, run-batched: route slots are
    permuted per expert (softmax over w is slot-invariant) so the
    route offset j-e is locally constant and one instruction covers
    several experts
  - e_w = exp(c*t):   ScalarE activation, scale=c_ew immediate, in-place
  - prod_w = e_w*Vs:  DVE, run-batched like t
  - den|num = sum_w:  DVE adds over a combined [e3|p3] layout - one add
    pass produces both reductions
  - r = 0.5/den = exp(-ln(den)+ln(.5)): ScalarE, func-clustered per
    group pair to limit ACT table reloads
  - out = num*r:      DVE mul
"""

import math

import numpy as np

import concourse.bass as bass
import concourse.mybir as mybir
from concourse import bacc, tile
from concourse.bass_utils import run_bass_kernel_spmd

E, NPROJ, B, P = 16, 2, 64, 4096
W = 3
EXPERT_DIM = 128
NCORES = 8
BS = B // NCORES          # 8 batch rows per core
COLS = 256                # free-dim columns per expert slab
PH = P // COLS            # 16 partition sub-blocks per batch row
PART = BS * PH            # 128 SBUF partitions
GROUP = 4                 # experts per tile group
NG = E // GROUP           # 4 groups
GC = GROUP * COLS         # 1024 cols per group tile
WGC = W * GC              # e3 / p3 section size in the combined tile

F32 = mybir.dt.float32
EXPF = mybir.ActivationFunctionType.Exp
LNF = mybir.ActivationFunctionType.Ln
ADD = mybir.AluOpType.add


def _runs(pairs):
    """Split [(le, j), ...] into maximal runs with consecutive le and j
    within one j-group."""
    runs = []
    for le, j in pairs:
        if (runs and runs[-1][0] + runs[-1][2] == le
                and runs[-1][1] + runs[-1][2] == j
                and (runs[-1][1] // GROUP == j // GROUP)):
            runs[-1][2] += 1
        else:
            runs.append([le, j, 1])
    return runs


def _build_nc(routes: np.ndarray, coef: np.ndarray):
    nc = bacc.Bacc("TRN2", target_bir_lowering=False, debug=False,
                   num_devices=NCORES)

    q_d = nc.dram_tensor("q", [E, NPROJ, BS, P], F32, kind="ExternalInput")
    k_d = nc.dram_tensor("k", [E, NPROJ, BS, P], F32, kind="ExternalInput")
    v_d = nc.dram_tensor("v", [E, NPROJ, BS, P], F32, kind="ExternalInput")
    o_d = nc.dram_tensor("out", [BS, E * P], F32, kind="ExternalOutput")

    # DRAM views: [(b ph), e, n, c]
    def lview(t):
        return t.ap().rearrange("e n b (ph c) -> (b ph) e n c", c=COLS)

    qv, kv, vv = lview(q_d), lview(k_d), lview(v_d)
    ov = o_d.ap().rearrange("b (e ph c) -> b ph e c", ph=PH, c=COLS)

    # group g of experts is ready once groups up to ready_g[g] are loaded
    ready_g = [max(g, int(routes[g * GROUP:(g + 1) * GROUP].max()) // GROUP)
               for g in range(NG)]

    with tile.TileContext(nc) as tc:
        with (
            tc.tile_pool(name="raw", bufs=4) as raw_p,
            tc.tile_pool(name="qs", bufs=NG) as qs_p,
            tc.tile_pool(name="ks", bufs=NG) as ks_p,
            tc.tile_pool(name="vs", bufs=NG) as vs_p,
            tc.tile_pool(name="tp", bufs=2) as tp_p,
            tc.tile_pool(name="dn", bufs=NG) as dn_p,
            tc.tile_pool(name="sm", bufs=2) as sm_p,
        ):
            qs, ks, vs = [], [], []

            def emit_phase1(g):
                """t, exp, prod for expert group g into a combined tile
                tp = [w0e3|w1e3|w2e3 | w0p3|w1p3|w2p3]."""
                e0 = g * GROUP
                tp = tp_p.tile([PART, 2 * WGC], F32, name="tp", tag="tp")
                # scaled scores: one stt per (e,w) (coef is per-instruction)
                for w in range(W):
                    for le in range(GROUP):
                        j = int(routes[e0 + le, w])
                        gj, lj = j // GROUP, j % GROUP
                        sl = slice(w * GC + le * COLS,
                                   w * GC + (le + 1) * COLS)
                        nc.vector.scalar_tensor_tensor(
                            tp[:, sl],
                            qs[g][:, le * COLS:(le + 1) * COLS],
                            float(coef[e0 + le, w]),
                            ks[gj][:, lj * COLS:(lj + 1) * COLS],
                            mybir.AluOpType.mult, mybir.AluOpType.mult)
                # one big exp per group half (scale=1, Exp table only)
                nc.scalar.activation(tp[:, 0:WGC // 2], tp[:, 0:WGC // 2],
                                     EXPF)
                nc.scalar.activation(tp[:, WGC // 2:WGC],
                                     tp[:, WGC // 2:WGC], EXPF)
                for w in range(W):
                    pairs = [(le, int(routes[e0 + le, w]))
                             for le in range(GROUP)]
                    for le, j, L in _runs(pairs):
                        gj, lj = j // GROUP, j % GROUP
                        nc.vector.tensor_mul(
                            tp[:, WGC + w * GC + le * COLS:
                               WGC + w * GC + (le + L) * COLS],
                            tp[:, w * GC + le * COLS:
                               w * GC + (le + L) * COLS],
                            vs[gj][:, lj * COLS:(lj + L) * COLS])
                # dn = [den | num], both w-sums in one add pass; frees tp
                dn = dn_p.tile([PART, 2 * GC], F32, name="dn", tag="dn")
                iv = [tp[:].rearrange("p (k w c) -> p k w c", k=2, w=W)
                      [:, :, w, :] for w in range(W)]
                dnv = dn[:].rearrange("p (k c) -> p k c", k=2)
                nc.vector.tensor_add(dnv, iv[0], iv[1])
                nc.vector.tensor_add(dnv, dnv, iv[2])
                return dn

            def emit_finale(g, dn):
                """recip / out / stores for one group (all DVE + stores)."""
                rcp = sm_p.tile([PART, GC], F32, name="rcp", tag="rcp")
                nc.vector.reciprocal_approx_fast(rcp[:], dn[:, 0:GC])
                og = sm_p.tile([PART, GC], F32, name="og", tag="og")
                nc.vector.scalar_tensor_tensor(
                    og[:], dn[:, GC:2 * GC], 0.5, rcp[:],
                    mybir.AluOpType.mult, mybir.AluOpType.mult)
                for le in range(GROUP):
                    ring().dma_start(ov[:, :, g * GROUP + le],
                                     og[:, le * COLS:(le + 1) * COLS])

            qs, ks, vs = [None] * NG, [None] * NG, [None] * NG
            # wave-gating: non-critical load DMAs wait for the loads that
            # unblock the first compute groups, so the SDMA engines aren't
            # round-robining against them on the critical path.
            gate_insts = []
            rings = [nc.sync, nc.scalar]
            ring_i = [0]

            def ring():
                ring_i[0] += 1
                return rings[ring_i[0] % 2]

            def gate(inst):
                for gi in gate_insts:
                    tile.add_dep_helper(inst.ins, gi.ins, sync=True,
                                        reason="load wave gating")

            def load_plain(dview, sums, s_p, g, wave1):
                """Plain load of both projections + DVE averaging."""
                es = slice(g * GROUP, (g + 1) * GROUP)
                s = s_p.tile([PART, GC], F32, name="s", tag="s")
                sv = s[:].rearrange("p (e c) -> p e c", e=GROUP)
                raw = raw_p.tile([PART, NPROJ * GC], F32, name="raw",
                                 tag="raw")
                rv = raw[:].rearrange("p (e n c) -> p e n c",
                                      e=GROUP, n=NPROJ)
                nh = GROUP // 2 if wave1 else GROUP
                for h0 in range(0, GROUP, nh):
                    hs = slice(g * GROUP + h0, g * GROUP + h0 + nh)
                    i0 = ring().dma_start(rv[:, h0:h0 + nh], dview[:, hs])
                    if wave1:
                        gate_insts.append(i0)
                    nc.vector.tensor_add(sv[:, h0:h0 + nh],
                                         rv[:, h0:h0 + nh, 0],
                                         rv[:, h0:h0 + nh, 1])
                sums[g] = s

            def load_accum(dview, sums, s_p, g):
                """Plain proj0 + DMA-accumulate proj1 (latency hides)."""
                es = slice(g * GROUP, (g + 1) * GROUP)
                s = s_p.tile([PART, GC], F32, name="s", tag="s")
                gate(ring().dma_start(s[:], dview[:, es, 0]))
                nc.gpsimd.dma_start(s[:], dview[:, es, 1], accum_op=ADD)
                sums[g] = s

            def load(dview, sums, s_p, g, wave1, is_v=False):
                if g >= 3 or (is_v and g >= 2):
                    load_accum(dview, sums, s_p, g)
                else:
                    load_plain(dview, sums, s_p, g, wave1)

            # Need-driven load order: emit exactly what unblocks the next
            # group's compute; late groups use DMA-accumulate.
            order = sorted(range(NG), key=lambda g: (ready_g[g], g))
            first = order[0]
            for g in order:
                wave1 = g == first
                rg = routes[g * GROUP:(g + 1) * GROUP]
                kneed = sorted({int(j) // GROUP for j in rg.flatten()})
                if qs[g] is None:
                    load(qv, qs, qs_p, g, wave1)
                for gj in kneed:
                    if ks[gj] is None:
                        load(kv, ks, ks_p, gj, wave1)
                for gj in kneed:
                    if vs[gj] is None:
                        load(vv, vs, vs_p, gj, False, is_v=True)
                dn = emit_phase1(g)
                emit_finale(g, dn)

    nc.compile()
    return nc


_cache: dict = {}


def _get_nc(routes: np.ndarray, coef: np.ndarray):
    key = (routes.tobytes(), coef.tobytes())
    if key not in _cache:
        _cache[key] = _build_nc(routes, coef)
    return _cache[key]


def kernel(Q_proj, K_proj, V_proj, betas, temperature, routes, num_patches):
    Q = np.asarray(Q_proj, dtype=np.float32)
    K = np.asarray(K_proj, dtype=np.float32)
    V = np.asarray(V_proj, dtype=np.float32)
    betas = np.asarray(betas, dtype=np.float32)
    temp = np.asarray(temperature, dtype=np.float32)
    routes = np.asarray(routes, dtype=np.int32)

    # Host control-plane: beta gating + scale folded into one coefficient
    # per (expert, neighbor).  0.25 = the two projection means of Q and K
    # (sums are averaged); V's 0.5 is folded into the reciprocal's bias.
    scale = np.float32(np.sqrt(np.float32(EXPERT_DIM))) * np.abs(temp[0])
    gate = np.where(routes != np.arange(E, dtype=np.int32)[:, None],
                    np.float32(1.0) / (np.float32(1.0) + np.exp(-betas)),
                    np.float32(1.0)).astype(np.float32)
    coef = (np.float32(0.25) * gate / scale).astype(np.float32)

    # Permute each expert's route slots so the offset j-e is sorted:
    # softmax over w is slot-invariant, and locally-constant offsets let
    # the builder batch consecutive experts into single instructions.
    order = np.argsort(routes - np.arange(E, dtype=np.int32)[:, None],
                       axis=1, kind="stable")
    routes_p = np.take_along_axis(routes, order, axis=1)
    coef_p = np.take_along_axis(coef, order, axis=1)

    nc = _get_nc(routes_p, coef_p)
    in_maps = [
        {
            "q": np.ascontiguousarray(Q[:, :, c * BS:(c + 1) * BS, :]),
            "k": np.ascontiguousarray(K[:, :, c * BS:(c + 1) * BS, :]),
            "v": np.ascontiguousarray(V[:, :, c * BS:(c + 1) * BS, :]),
        }
        for c in range(NCORES)
    ]
    res = run_bass_kernel_spmd(nc, in_maps, list(range(NCORES)))
    return np.concatenate([res.results[c]["out"] for c in range(NCORES)],
                          axis=0)


# revision 28
# speedup vs baseline: 1.0536x; 1.0536x over previous
"""Cantor global attention kernel for Trainium2 (8 NeuronCores, SPMD).

Strategy: data-parallel over batch B=64 -> 8 cores x 8 rows each.
Per core, every expert slab [8, 4096] is flattened to SBUF [128, 256]
(partition = b*16 + p//256, col = p%256); experts sit side by side in
the free dimension, grouped 4 per tile.  The W=3 neighbor gather and
the beta/temperature gating are folded into per-(e,w) instruction
operand offsets and exp-activation scale immediates, baked at build
time from the runtime routes/betas/temperature values (tiny [16,3]
control-plane tensors).

Engine placement (per core, f32).  DVE 2-operand ops and GpSimd ops
serialize on the shared SBUF port pair, so GpSimd runs NO tensor ops -
only SWDGE descriptor generation:
  - projection averaging (all of Q,K,V): DMA-accumulate (CCE add in
    the SDMA engines) - zero compute-engine cost
  - t_w = Qs*Ks:      DVE tensor_mul, run-batched: route slots are
    permuted per expert (softmax over w is slot-invariant) so the
    route offset j-e is locally constant and one instruction covers
    several experts
  - e_w = exp(c*t):   ScalarE activation, scale=c_ew immediate, in-place
  - prod_w = e_w*Vs:  DVE, run-batched like t
  - den|num = sum_w:  DVE adds over a combined [e3|p3] layout - one add
    pass produces both reductions
  - r = 0.5/den = exp(-ln(den)+ln(.5)): ScalarE, func-clustered per
    group pair to limit ACT table reloads
  - out = num*r:      DVE mul
"""

import math

import numpy as np

import concourse.bass as bass
import concourse.mybir as mybir
from concourse import bacc, tile
from concourse.bass_utils import run_bass_kernel_spmd

E, NPROJ, B, P = 16, 2, 64, 4096
W = 3
EXPERT_DIM = 128
NCORES = 8
BS = B // NCORES          # 8 batch rows per core
COLS = 256                # free-dim columns per expert slab
PH = P // COLS            # 16 partition sub-blocks per batch row
PART = BS * PH            # 128 SBUF partitions
GROUP = 4                 # experts per tile group
NG = E // GROUP           # 4 groups
GC = GROUP * COLS         # 1024 cols per group tile
WGC = W * GC              # e3 / p3 section size in the combined tile

F32 = mybir.dt.float32
EXPF = mybir.ActivationFunctionType.Exp
LNF = mybir.ActivationFunctionType.Ln
ADD = mybir.AluOpType.add


def _runs(pairs):
    """Split [(le, j), ...] into maximal runs with consecutive le and j
    within one j-group."""
    runs = []
    for le, j in pairs:
        if (runs and runs[-1][0] + runs[-1][2] == le
                and runs[-1][1] + runs[-1][2] == j
                and (runs[-1][1] // GROUP == j // GROUP)):
            runs[-1][2] += 1
        else:
            runs.append([le, j, 1])
    return runs


def _build_nc(routes: np.ndarray, coef: np.ndarray):
    nc = bacc.Bacc("TRN2", target_bir_lowering=False, debug=False,
                   num_devices=NCORES)

    q_d = nc.dram_tensor("q", [E, NPROJ, BS, P], F32, kind="ExternalInput")
    k_d = nc.dram_tensor("k", [E, NPROJ, BS, P], F32, kind="ExternalInput")
    v_d = nc.dram_tensor("v", [E, NPROJ, BS, P], F32, kind="ExternalInput")
    o_d = nc.dram_tensor("out", [BS, E * P], F32, kind="ExternalOutput")

    # DRAM views: [(b ph), e, n, c]
    def lview(t):
        return t.ap().rearrange("e n b (ph c) -> (b ph) e n c", c=COLS)

    qv, kv, vv = lview(q_d), lview(k_d), lview(v_d)
    ov = o_d.ap().rearrange("b (e ph c) -> b ph e c", ph=PH, c=COLS)

    # group g of experts is ready once groups up to ready_g[g] are loaded
    ready_g = [max(g, int(routes[g * GROUP:(g + 1) * GROUP].max()) // GROUP)
               for g in range(NG)]

    with tile.TileContext(nc) as tc:
        with (
            tc.tile_pool(name="raw", bufs=4) as raw_p,
            tc.tile_pool(name="qs", bufs=NG) as qs_p,
            tc.tile_pool(name="ks", bufs=NG) as ks_p,
            tc.tile_pool(name="vs", bufs=NG) as vs_p,
            tc.tile_pool(name="tp", bufs=2) as tp_p,
            tc.tile_pool(name="dn", bufs=NG) as dn_p,
            tc.tile_pool(name="sm", bufs=2) as sm_p,
        ):
            qs, ks, vs = [], [], []

            def emit_phase1(g):
                """t, exp, prod for expert group g into a combined tile
                tp = [w0e3|w1e3|w2e3 | w0p3|w1p3|w2p3]."""
                e0 = g * GROUP
                tp = tp_p.tile([PART, 2 * WGC], F32, name="tp", tag="tp")
                for w in range(W):
                    pairs = [(le, int(routes[e0 + le, w]))
                             for le in range(GROUP)]
                    for le, j, L in _runs(pairs):
                        gj, lj = j // GROUP, j % GROUP
                        nc.vector.tensor_mul(
                            tp[:, w * GC + le * COLS:
                               w * GC + (le + L) * COLS],
                            qs[g][:, le * COLS:(le + L) * COLS],
                            ks[gj][:, lj * COLS:(lj + L) * COLS])
                    for le in range(GROUP):
                        sl = slice(w * GC + le * COLS,
                                   w * GC + (le + 1) * COLS)
                        nc.scalar.activation(tp[:, sl], tp[:, sl], EXPF,
                                             bias=0.0,
                                             scale=float(coef[e0 + le, w]))
                for w in range(W):
                    pairs = [(le, int(routes[e0 + le, w]))
                             for le in range(GROUP)]
                    for le, j, L in _runs(pairs):
                        gj, lj = j // GROUP, j % GROUP
                        nc.vector.tensor_mul(
                            tp[:, WGC + w * GC + le * COLS:
                               WGC + w * GC + (le + L) * COLS],
                            tp[:, w * GC + le * COLS:
                               w * GC + (le + L) * COLS],
                            vs[gj][:, lj * COLS:(lj + L) * COLS])
                # dn = [den | num], both w-sums in one add pass; frees tp
                dn = dn_p.tile([PART, 2 * GC], F32, name="dn", tag="dn")
                iv = [tp[:].rearrange("p (k w c) -> p k w c", k=2, w=W)
                      [:, :, w, :] for w in range(W)]
                dnv = dn[:].rearrange("p (k c) -> p k c", k=2)
                nc.vector.tensor_add(dnv, iv[0], iv[1])
                nc.vector.tensor_add(dnv, dnv, iv[2])
                return dn

            def emit_finale(g, dn):
                """recip / out / stores for one group (all DVE + stores)."""
                rcp = sm_p.tile([PART, GC], F32, name="rcp", tag="rcp")
                nc.vector.reciprocal_approx_fast(rcp[:], dn[:, 0:GC])
                og = sm_p.tile([PART, GC], F32, name="og", tag="og")
                nc.vector.scalar_tensor_tensor(
                    og[:], dn[:, GC:2 * GC], 0.5, rcp[:],
                    mybir.AluOpType.mult, mybir.AluOpType.mult)
                for le in range(GROUP):
                    nc.sync.dma_start(ov[:, :, g * GROUP + le],
                                      og[:, le * COLS:(le + 1) * COLS])

            qs, ks, vs = [None] * NG, [None] * NG, [None] * NG
            # wave-gating: non-critical load DMAs wait for the loads that
            # unblock the first compute groups, so the SDMA engines aren't
            # round-robining against them on the critical path.
            gate_insts = []
            rings = [nc.sync, nc.scalar]
            ring_i = [0]

            def ring():
                ring_i[0] += 1
                return rings[ring_i[0] % 2]

            def gate(inst):
                for gi in gate_insts:
                    tile.add_dep_helper(inst.ins, gi.ins, sync=True,
                                        reason="load wave gating")

            def load_plain(dview, sums, s_p, g, wave1):
                """Plain load of both projections + DVE averaging."""
                es = slice(g * GROUP, (g + 1) * GROUP)
                s = s_p.tile([PART, GC], F32, name="s", tag="s")
                sv = s[:].rearrange("p (e c) -> p e c", e=GROUP)
                raw = raw_p.tile([PART, NPROJ * GC], F32, name="raw",
                                 tag="raw")
                rv = raw[:].rearrange("p (e n c) -> p e n c",
                                      e=GROUP, n=NPROJ)
                nh = GROUP // 2 if wave1 else GROUP
                for h0 in range(0, GROUP, nh):
                    hs = slice(g * GROUP + h0, g * GROUP + h0 + nh)
                    i0 = ring().dma_start(rv[:, h0:h0 + nh], dview[:, hs])
                    if wave1:
                        gate_insts.append(i0)
                    else:
                        gate(i0)
                    nc.vector.tensor_add(sv[:, h0:h0 + nh],
                                         rv[:, h0:h0 + nh, 0],
                                         rv[:, h0:h0 + nh, 1])
                sums[g] = s

            def load_accum(dview, sums, s_p, g):
                """Plain proj0 + DMA-accumulate proj1 (latency hides)."""
                es = slice(g * GROUP, (g + 1) * GROUP)
                s = s_p.tile([PART, GC], F32, name="s", tag="s")
                gate(ring().dma_start(s[:], dview[:, es, 0]))
                nc.gpsimd.dma_start(s[:], dview[:, es, 1], accum_op=ADD)
                sums[g] = s

            def load(dview, sums, s_p, g, wave1, is_v=False):
                if g >= 2:
                    load_accum(dview, sums, s_p, g)
                else:
                    load_plain(dview, sums, s_p, g, wave1)

            # Need-driven load order: emit exactly what unblocks the next
            # group's compute; late groups use DMA-accumulate.
            order = sorted(range(NG), key=lambda g: (ready_g[g], g))
            first = order[0]
            for g in order:
                wave1 = g == first
                rg = routes[g * GROUP:(g + 1) * GROUP]
                kneed = sorted({int(j) // GROUP for j in rg.flatten()})
                if qs[g] is None:
                    load(qv, qs, qs_p, g, wave1)
                for gj in kneed:
                    if ks[gj] is None:
                        load(kv, ks, ks_p, gj, wave1)
                for gj in kneed:
                    if vs[gj] is None:
                        load(vv, vs, vs_p, gj, False, is_v=True)
                dn = emit_phase1(g)
                emit_finale(g, dn)

    nc.compile()
    return nc


_cache: dict = {}


def _get_nc(routes: np.ndarray, coef: np.ndarray):
    key = (routes.tobytes(), coef.tobytes())
    if key not in _cache:
        _cache[key] = _build_nc(routes, coef)
    return _cache[key]


def kernel(Q_proj, K_proj, V_proj, betas, temperature, routes, num_patches):
    Q = np.asarray(Q_proj, dtype=np.float32)
    K = np.asarray(K_proj, dtype=np.float32)
    V = np.asarray(V_proj, dtype=np.float32)
    betas = np.asarray(betas, dtype=np.float32)
    temp = np.asarray(temperature, dtype=np.float32)
    routes = np.asarray(routes, dtype=np.int32)

    # Host control-plane: beta gating + scale folded into one coefficient
    # per (expert, neighbor).  0.25 = the two projection means of Q and K
    # (sums are averaged); V's 0.5 is folded into the reciprocal's bias.
    scale = np.float32(np.sqrt(np.float32(EXPERT_DIM))) * np.abs(temp[0])
    gate = np.where(routes != np.arange(E, dtype=np.int32)[:, None],
                    np.float32(1.0) / (np.float32(1.0) + np.exp(-betas)),
                    np.float32(1.0)).astype(np.float32)
    coef = (np.float32(0.25) * gate / scale).astype(np.float32)

    # Permute each expert's route slots so the offset j-e is sorted:
    # softmax over w is slot-invariant, and locally-constant offsets let
    # the builder batch consecutive experts into single instructions.
    order = np.argsort(routes - np.arange(E, dtype=np.int32)[:, None],
                       axis=1, kind="stable")
    routes_p = np.take_along_axis(routes, order, axis=1)
    coef_p = np.take_along_axis(coef, order, axis=1)

    nc = _get_nc(routes_p, coef_p)
    in_maps = [
        {
            "q": np.ascontiguousarray(Q[:, :, c * BS:(c + 1) * BS, :]),
            "k": np.ascontiguousarray(K[:, :, c * BS:(c + 1) * BS, :]),
            "v": np.ascontiguousarray(V[:, :, c * BS:(c + 1) * BS, :]),
        }
        for c in range(NCORES)
    ]
    res = run_bass_kernel_spmd(nc, in_maps, list(range(NCORES)))
    return np.concatenate([res.results[c]["out"] for c in range(NCORES)],
                          axis=0)


# revision 29
# speedup vs baseline: 1.0545x; 1.0008x over previous
"""Cantor global attention kernel for Trainium2 (8 NeuronCores, SPMD).

Strategy: data-parallel over batch B=64 -> 8 cores x 8 rows each.
Per core, every expert slab [8, 4096] is flattened to SBUF [128, 256]
(partition = b*16 + p//256, col = p%256); experts sit side by side in
the free dimension, grouped 4 per tile.  The W=3 neighbor gather and
the beta/temperature gating are folded into per-(e,w) instruction
operand offsets and exp-activation scale immediates, baked at build
time from the runtime routes/betas/temperature values (tiny [16,3]
control-plane tensors).

Engine placement (per core, f32).  DVE 2-operand ops and GpSimd ops
serialize on the shared SBUF port pair, so GpSimd runs NO tensor ops -
only SWDGE descriptor generation:
  - projection averaging (all of Q,K,V): DMA-accumulate (CCE add in
    the SDMA engines) - zero compute-engine cost
  - t_w = Qs*Ks:      DVE tensor_mul, run-batched: route slots are
    permuted per expert (softmax over w is slot-invariant) so the
    route offset j-e is locally constant and one instruction covers
    several experts
  - e_w = exp(c*t):   ScalarE activation, scale=c_ew immediate, in-place
  - prod_w = e_w*Vs:  DVE, run-batched like t
  - den|num = sum_w:  DVE adds over a combined [e3|p3] layout - one add
    pass produces both reductions
  - r = 0.5/den = exp(-ln(den)+ln(.5)): ScalarE, func-clustered per
    group pair to limit ACT table reloads
  - out = num*r:      DVE mul
"""

import math

import numpy as np

import concourse.bass as bass
import concourse.mybir as mybir
from concourse import bacc, tile
from concourse.bass_utils import run_bass_kernel_spmd

E, NPROJ, B, P = 16, 2, 64, 4096
W = 3
EXPERT_DIM = 128
NCORES = 8
BS = B // NCORES          # 8 batch rows per core
COLS = 256                # free-dim columns per expert slab
PH = P // COLS            # 16 partition sub-blocks per batch row
PART = BS * PH            # 128 SBUF partitions
GROUP = 4                 # experts per tile group
NG = E // GROUP           # 4 groups
GC = GROUP * COLS         # 1024 cols per group tile
WGC = W * GC              # e3 / p3 section size in the combined tile

F32 = mybir.dt.float32
EXPF = mybir.ActivationFunctionType.Exp
LNF = mybir.ActivationFunctionType.Ln
ADD = mybir.AluOpType.add


def _runs(pairs):
    """Split [(le, j), ...] into maximal runs with consecutive le and j
    within one j-group."""
    runs = []
    for le, j in pairs:
        if (runs and runs[-1][0] + runs[-1][2] == le
                and runs[-1][1] + runs[-1][2] == j
                and (runs[-1][1] // GROUP == j // GROUP)):
            runs[-1][2] += 1
        else:
            runs.append([le, j, 1])
    return runs


def _build_nc(routes: np.ndarray, coef: np.ndarray):
    nc = bacc.Bacc("TRN2", target_bir_lowering=False, debug=False,
                   num_devices=NCORES)

    q_d = nc.dram_tensor("q", [E, NPROJ, BS, P], F32, kind="ExternalInput")
    k_d = nc.dram_tensor("k", [E, NPROJ, BS, P], F32, kind="ExternalInput")
    v_d = nc.dram_tensor("v", [E, NPROJ, BS, P], F32, kind="ExternalInput")
    o_d = nc.dram_tensor("out", [BS, E * P], F32, kind="ExternalOutput")

    # DRAM views: [(b ph), e, n, c]
    def lview(t):
        return t.ap().rearrange("e n b (ph c) -> (b ph) e n c", c=COLS)

    qv, kv, vv = lview(q_d), lview(k_d), lview(v_d)
    ov = o_d.ap().rearrange("b (e ph c) -> b ph e c", ph=PH, c=COLS)

    # group g of experts is ready once groups up to ready_g[g] are loaded
    ready_g = [max(g, int(routes[g * GROUP:(g + 1) * GROUP].max()) // GROUP)
               for g in range(NG)]

    with tile.TileContext(nc) as tc:
        with (
            tc.tile_pool(name="raw", bufs=4) as raw_p,
            tc.tile_pool(name="qs", bufs=NG) as qs_p,
            tc.tile_pool(name="ks", bufs=NG) as ks_p,
            tc.tile_pool(name="vs", bufs=NG) as vs_p,
            tc.tile_pool(name="tp", bufs=2) as tp_p,
            tc.tile_pool(name="dn", bufs=NG) as dn_p,
            tc.tile_pool(name="sm", bufs=2) as sm_p,
        ):
            qs, ks, vs = [], [], []

            def emit_phase1(g):
                """t, exp, prod for expert group g into a combined tile
                tp = [w0e3|w1e3|w2e3 | w0p3|w1p3|w2p3]."""
                e0 = g * GROUP
                tp = tp_p.tile([PART, 2 * WGC], F32, name="tp", tag="tp")
                for w in range(W):
                    pairs = [(le, int(routes[e0 + le, w]))
                             for le in range(GROUP)]
                    for le, j, L in _runs(pairs):
                        gj, lj = j // GROUP, j % GROUP
                        nc.vector.tensor_mul(
                            tp[:, w * GC + le * COLS:
                               w * GC + (le + L) * COLS],
                            qs[g][:, le * COLS:(le + L) * COLS],
                            ks[gj][:, lj * COLS:(lj + L) * COLS])
                    for le in range(GROUP):
                        sl = slice(w * GC + le * COLS,
                                   w * GC + (le + 1) * COLS)
                        nc.scalar.activation(tp[:, sl], tp[:, sl], EXPF,
                                             bias=0.0,
                                             scale=float(coef[e0 + le, w]))
                for w in range(W):
                    pairs = [(le, int(routes[e0 + le, w]))
                             for le in range(GROUP)]
                    for le, j, L in _runs(pairs):
                        gj, lj = j // GROUP, j % GROUP
                        nc.vector.tensor_mul(
                            tp[:, WGC + w * GC + le * COLS:
                               WGC + w * GC + (le + L) * COLS],
                            tp[:, w * GC + le * COLS:
                               w * GC + (le + L) * COLS],
                            vs[gj][:, lj * COLS:(lj + L) * COLS])
                # dn = [den | num], both w-sums in one add pass; frees tp
                dn = dn_p.tile([PART, 2 * GC], F32, name="dn", tag="dn")
                iv = [tp[:].rearrange("p (k w c) -> p k w c", k=2, w=W)
                      [:, :, w, :] for w in range(W)]
                dnv = dn[:].rearrange("p (k c) -> p k c", k=2)
                nc.vector.tensor_add(dnv, iv[0], iv[1])
                nc.vector.tensor_add(dnv, dnv, iv[2])
                return dn

            def emit_finale(g, dn):
                """recip / out / stores for one group (all DVE + stores).
                Halved so the first stores issue while the second half of
                the normalize still runs - shortens the kernel tail."""
                og = sm_p.tile([PART, GC], F32, name="og", tag="og")
                for h in range(2):
                    hc = slice(h * GC // 2, (h + 1) * GC // 2)
                    rcp = sm_p.tile([PART, GC // 2], F32, name="rcp",
                                    tag="rcp", bufs=4)
                    nc.vector.reciprocal_approx_fast(rcp[:], dn[:, hc])
                    nc.vector.scalar_tensor_tensor(
                        og[:, hc], dn[:, GC + h * GC // 2:
                                      GC + (h + 1) * GC // 2], 0.5, rcp[:],
                        mybir.AluOpType.mult, mybir.AluOpType.mult)
                    for le in range(h * GROUP // 2, (h + 1) * GROUP // 2):
                        nc.sync.dma_start(ov[:, :, g * GROUP + le],
                                          og[:, le * COLS:(le + 1) * COLS])

            qs, ks, vs = [None] * NG, [None] * NG, [None] * NG
            # wave-gating: non-critical load DMAs wait for the loads that
            # unblock the first compute groups, so the SDMA engines aren't
            # round-robining against them on the critical path.
            gate_insts = []
            rings = [nc.sync, nc.scalar]
            ring_i = [0]

            def ring():
                ring_i[0] += 1
                return rings[ring_i[0] % 2]

            def gate(inst):
                for gi in gate_insts:
                    tile.add_dep_helper(inst.ins, gi.ins, sync=True,
                                        reason="load wave gating")

            def load_plain(dview, sums, s_p, g, wave1):
                """Plain load of both projections + DVE averaging."""
                es = slice(g * GROUP, (g + 1) * GROUP)
                s = s_p.tile([PART, GC], F32, name="s", tag="s")
                sv = s[:].rearrange("p (e c) -> p e c", e=GROUP)
                raw = raw_p.tile([PART, NPROJ * GC], F32, name="raw",
                                 tag="raw")
                rv = raw[:].rearrange("p (e n c) -> p e n c",
                                      e=GROUP, n=NPROJ)
                nh = GROUP // 2 if wave1 else GROUP
                for h0 in range(0, GROUP, nh):
                    hs = slice(g * GROUP + h0, g * GROUP + h0 + nh)
                    i0 = ring().dma_start(rv[:, h0:h0 + nh], dview[:, hs])
                    if wave1:
                        gate_insts.append(i0)
                    else:
                        gate(i0)
                    nc.vector.tensor_add(sv[:, h0:h0 + nh],
                                         rv[:, h0:h0 + nh, 0],
                                         rv[:, h0:h0 + nh, 1])
                sums[g] = s

            def load_accum(dview, sums, s_p, g):
                """Plain proj0 + DMA-accumulate proj1 (latency hides)."""
                es = slice(g * GROUP, (g + 1) * GROUP)
                s = s_p.tile([PART, GC], F32, name="s", tag="s")
                gate(ring().dma_start(s[:], dview[:, es, 0]))
                nc.gpsimd.dma_start(s[:], dview[:, es, 1], accum_op=ADD)
                sums[g] = s

            def load(dview, sums, s_p, g, wave1, is_v=False):
                if g >= 2:
                    load_accum(dview, sums, s_p, g)
                else:
                    load_plain(dview, sums, s_p, g, wave1)

            # Need-driven load order: emit exactly what unblocks the next
            # group's compute; late groups use DMA-accumulate.
            order = sorted(range(NG), key=lambda g: (ready_g[g], g))
            first = order[0]
            for g in order:
                wave1 = g == first
                rg = routes[g * GROUP:(g + 1) * GROUP]
                kneed = sorted({int(j) // GROUP for j in rg.flatten()})
                if qs[g] is None:
                    load(qv, qs, qs_p, g, wave1)
                for gj in kneed:
                    if ks[gj] is None:
                        load(kv, ks, ks_p, gj, wave1)
                for gj in kneed:
                    if vs[gj] is None:
                        load(vv, vs, vs_p, gj, False, is_v=True)
                dn = emit_phase1(g)
                emit_finale(g, dn)

    nc.compile()
    return nc


_cache: dict = {}


def _get_nc(routes: np.ndarray, coef: np.ndarray):
    key = (routes.tobytes(), coef.tobytes())
    if key not in _cache:
        _cache[key] = _build_nc(routes, coef)
    return _cache[key]


def kernel(Q_proj, K_proj, V_proj, betas, temperature, routes, num_patches):
    Q = np.asarray(Q_proj, dtype=np.float32)
    K = np.asarray(K_proj, dtype=np.float32)
    V = np.asarray(V_proj, dtype=np.float32)
    betas = np.asarray(betas, dtype=np.float32)
    temp = np.asarray(temperature, dtype=np.float32)
    routes = np.asarray(routes, dtype=np.int32)

    # Host control-plane: beta gating + scale folded into one coefficient
    # per (expert, neighbor).  0.25 = the two projection means of Q and K
    # (sums are averaged); V's 0.5 is folded into the reciprocal's bias.
    scale = np.float32(np.sqrt(np.float32(EXPERT_DIM))) * np.abs(temp[0])
    gate = np.where(routes != np.arange(E, dtype=np.int32)[:, None],
                    np.float32(1.0) / (np.float32(1.0) + np.exp(-betas)),
                    np.float32(1.0)).astype(np.float32)
    coef = (np.float32(0.25) * gate / scale).astype(np.float32)

    # Permute each expert's route slots so the offset j-e is sorted:
    # softmax over w is slot-invariant, and locally-constant offsets let
    # the builder batch consecutive experts into single instructions.
    order = np.argsort(routes - np.arange(E, dtype=np.int32)[:, None],
                       axis=1, kind="stable")
    routes_p = np.take_along_axis(routes, order, axis=1)
    coef_p = np.take_along_axis(coef, order, axis=1)

    nc = _get_nc(routes_p, coef_p)
    in_maps = [
        {
            "q": np.ascontiguousarray(Q[:, :, c * BS:(c + 1) * BS, :]),
            "k": np.ascontiguousarray(K[:, :, c * BS:(c + 1) * BS, :]),
            "v": np.ascontiguousarray(V[:, :, c * BS:(c + 1) * BS, :]),
        }
        for c in range(NCORES)
    ]
    res = run_bass_kernel_spmd(nc, in_maps, list(range(NCORES)))
    return np.concatenate([res.results[c]["out"] for c in range(NCORES)],
                          axis=0)
